# revision 25
# baseline (speedup 1.0000x reference)
"""Trainium2 Bass kernel for nn_MemoryAugmentedModel (gnn_message_passing).

Math: the reference only consumes row N-1 of the GAT output, so the dense
[N,N,H] attention collapses:
  out[-1] = (1/H) * sum_h gat_w_h @ (softmax_j(lrelu(a_dst[-1,h]+a_src[j,h])) @ nf) + gat_bias
with a_src = nf @ V_src^T, V_src[h] = att_src[h] @ gat_w_h  (same for dst).
Then LayerNorm -> proj/LoRA offset -> embedding gather with offset added to
each sequence's first token.

Sharding (8 cores): gat_w / node_features split by input-feature columns
(e-chunks of 256) -> partial attention logits (AllReduce #1, bf16 [128,68])
-> replicated softmax -> per-core agg over its e-chunk -> partial out[-1]
(AllReduce #2, f32 [1,2176] with ridden row-sum) -> replicated LayerNorm ->
proj/LoRA offset chunk [1,256] added directly to column-gathered first-token
embedding pieces [4,256] (aux output; host stitches) -- no AllGather needed.
Each core also gathers 1024 of the 8192 output embedding rows.

Schedule: a warmup AllReduce at t=0 absorbs the first-collective channel
init. Chain-critical inputs (att, w_nat, nf) load on the sync queue (then
the sync queue takes the gather writeback); bulk post-AR1 weights (w_tr,
proj, lora) plus the post-collective readbacks and activations run on the
scalar queue; the embedding gather is gated behind AR1's trigger in gpsimd
program order so it never starves the serial chain.
"""

import os
import sys
import types

import numpy as np

NCORES = 8
N = 2048
D = 2048
H = 4
R = 32
V = 32000
B = 4
S = 2048

EC = D // NCORES          # 256: e-columns (input features) per core
FC = D // NCORES          # 256: offset cols per core
ROWS = (B * S) // NCORES  # 1024: output embedding rows per core
NG = ROWS // 128          # 8 gather groups per core
NU = D // 128             # 16: 128-row chunks of a length-D axis
NT = (H * D) // 128       # 64: 128-row strips of gat_w
AR2W = 17 * 128           # 2176: AR2 payload (2048 row + sum + pad)

_CACHE = {}


def _install_ntff_shim():
    """Register the axon NTFF profile hook missing from this image's antenv."""
    if "antenv.axon_hooks" in sys.modules:
        return
    try:
        import antenv
        from trn_agent_boot.trn_boot import _ntff_profile_via_ctypes
    except Exception:
        return
    mod = types.ModuleType("antenv.axon_hooks")
    mod._hook = None
    mod.set_axon_ntff_profile_hook = lambda h: setattr(mod, "_hook", h)
    mod.get_axon_ntff_profile_hook = lambda: mod._hook
    sys.modules["antenv.axon_hooks"] = mod
    antenv.axon_hooks = mod
    try:
        mod.set_axon_ntff_profile_hook(
            _ntff_profile_via_ctypes("/opt/axon/libaxon_pjrt.so")
        )
    except Exception:
        pass


def _build():
    import concourse.bacc as bacc
    import concourse.bass as bass
    import concourse.tile as tile
    from concourse import mybir

    f32 = mybir.dt.float32
    bf16 = mybir.dt.bfloat16
    fp8 = mybir.dt.float8e4
    i32 = mybir.dt.int32
    RG = [list(range(NCORES))]
    AT = mybir.AluOpType

    nc = bacc.Bacc("TRN2", target_bir_lowering=False, debug=False,
                   num_devices=NCORES)

    din = lambda name, shape, dt: nc.dram_tensor(name, shape, dt, kind="ExternalInput").ap()
    w_nat = din("w_nat", [128, NT, EC], bf16)
    w_tr = din("w_tr", [2 * 128, H * D], bf16)
    att_st = din("att_st", [128, NT, 2 * H], bf16)   # zero-padded per strip
    nf_pre = din("nf_pre", [128, NU, EC], bf16)
    nf_tr = din("nf_tr", [2 * 128, N], bf16)
    proj_pre = din("proj_pre", [128, NU, FC], bf16)
    projb_r = din("projb_r", [1, FC], f32)
    lora_a_pre = din("lora_a_pre", [128, NU, R], bf16)
    lora_bt = din("lora_bt", [R, FC], bf16)
    gbias_row = din("gbias_row", [1, D], f32)
    gamma_r = din("gamma_r", [128, NU], f32)
    beta_r = din("beta_r", [128, NU], f32)
    ids_r = din("ids_r", [128, NG], i32)
    ids_ft = din("ids_ft", [4, 1], i32)   # first-token ids, pre-scaled *8+core
    embed = din("embed", [V, D], f32)

    out_sl = nc.dram_tensor("out_sl", [ROWS, D], f32, kind="ExternalOutput").ap()
    ft_out = nc.dram_tensor("ft_out", [4, FC], f32, kind="ExternalOutput").ap()

    with tile.TileContext(nc) as tc:
        import contextlib
        ctx = contextlib.ExitStack()
        with ctx:
            const = ctx.enter_context(tc.tile_pool(name="const", bufs=1))
            embp = ctx.enter_context(tc.tile_pool(name="embp", bufs=NG))
            dram = ctx.enter_context(tc.tile_pool(name="dram", bufs=1, space="DRAM"))

            ids_st = const.tile([128, NG], i32)
            nc.gpsimd.dma_start(ids_st[:], ids_r[:])
            idft_sb = const.tile([4, 1], i32)
            nc.gpsimd.dma_start(idft_sb[:], ids_ft[:])

            # ---- chain-critical inputs split across both HW queues -----------
            attst_sb = const.tile([128, NT, 2 * H], bf16)
            nc.sync.dma_start(attst_sb[:], att_st[:])
            wn_sb = const.tile([128, NT, EC], bf16)
            for ch in range(4):
                nc.sync.dma_start(wn_sb[:, ch * 16:(ch + 1) * 16, :],
                                  w_nat[:, ch * 16:(ch + 1) * 16, :])
            nft_sb = []
            for half in range(2):
                t = const.tile([128, N], bf16, name=f"nft{half}", tag=f"nft{half}")
                nc.scalar.dma_start(t[:], nf_tr[half * 128:(half + 1) * 128, :])
                nft_sb.append(t)
            nf_sb = const.tile([128, NU, EC + 1], bf16)
            nc.scalar.dma_start(nf_sb[:, :, 0:EC], nf_pre[:])
            nc.vector.memset(nf_sb[:, :, EC:EC + 1], 1.0)

            # ---- bulk post-AR1 weights behind the chain inputs on sync -------
            wt_sb = []
            for half in range(2):
                t = const.tile([128, H * D], bf16, name=f"wt{half}", tag=f"wt{half}")
                nc.sync.dma_start(t[:], w_tr[half * 128:(half + 1) * 128, :])
                wt_sb.append(t)
            proj_sb = const.tile([128, NU, FC], bf16)
            nc.sync.dma_start(proj_sb[:], proj_pre[:])
            projb_sb = const.tile([1, FC], f32)
            nc.sync.dma_start(projb_sb[:], projb_r[:])
            lat_sb = const.tile([128, NU, R], bf16)
            nc.sync.dma_start(lat_sb[:], lora_a_pre[:])
            lbt_sb = const.tile([R, FC], bf16)
            nc.sync.dma_start(lbt_sb[:], lora_bt[:])
            gbrow_sb = const.tile([1, D], f32)
            nc.sync.dma_start(gbrow_sb[:], gbias_row[:])
            gamma_sb = const.tile([128, NU], f32)
            nc.sync.dma_start(gamma_sb[:], gamma_r[:])
            beta_sb = const.tile([128, NU], f32)
            nc.sync.dma_start(beta_sb[:], beta_r[:])
            ident_sb = const.tile([128, 128], bf16)
            from concourse.masks import make_identity
            make_identity(nc, ident_sb[:])
            ones1_sb = const.tile([1, 128], f32)
            nc.vector.memset(ones1_sb[:], 1.0)
            ones1b_sb = const.tile([1, 128], bf16)
            nc.vector.memset(ones1b_sb[:], 1.0)
            onescol_sb = const.tile([128, 1], f32)
            nc.vector.memset(onescol_sb[:], 1.0)
            # preload the activation table so exp doesn't pay for it mid-chain
            dummy_sb = const.tile([1, 1], f32)
            nc.vector.memset(dummy_sb[:], 0.0)
            nc.scalar.activation(out=dummy_sb[:], in_=dummy_sb[:],
                                 func=mybir.ActivationFunctionType.Exp)

            # gate the embedding gather on the chain inputs having landed:
            # ids_sb = ids_st + (0 derived from wn/nf), so the indirect DMAs
            # cannot be scheduled before the attention-path inputs are in SBUF
            gatef_sb = const.tile([1, 1], f32)
            nc.vector.tensor_tensor(out=gatef_sb[:], in0=wn_sb[0:1, NT - 1, EC - 1:EC],
                                    in1=nf_sb[0:1, NU - 1, EC:EC + 1], op=AT.mult)
            gcol_sb = const.tile([128, 1], i32)
            with tc.tile_pool(name="ppz", bufs=1, space="PSUM") as ppz:
                ps_z = ppz.tile([128, 1], f32, tag="z")
                nc.tensor.matmul(out=ps_z[:], lhsT=ones1_sb[:], rhs=gatef_sb[:],
                                 start=True, stop=True)
                nc.vector.tensor_scalar_mul(gcol_sb[:], ps_z[:], 0.0)
            ids_sb = const.tile([128, NG], i32)
            gz_b = bass.AP(tensor=gcol_sb[:].tensor, offset=gcol_sb[:].offset,
                           ap=[gcol_sb[:].ap[0], [0, NG]])
            nc.vector.tensor_tensor(out=ids_sb[:], in0=ids_st[:], in1=gz_b,
                                    op=AT.add)

            # ---- embedding gather (gens precede the CC triggers on gpsimd) ---
            emb_tiles = []
            for g in range(NG):
                et = embp.tile([128, D], f32, name=f"emb{g}", tag="emb")
                nc.gpsimd.indirect_dma_start(
                    out=et[:], out_offset=None, in_=embed[:, :],
                    in_offset=bass.IndirectOffsetOnAxis(ap=ids_sb[:, g:g + 1], axis=0),
                )
                emb_tiles.append(et)
                nc.sync.dma_start(out_sl[g * 128:(g + 1) * 128, :], et[:])
            # first-token embedding column pieces [4, FC] for this core's chunk
            # (ids pre-scaled host-side: idx = id*8 + core, over a [V*8, FC] view)
            emb8 = embed[:, :].rearrange("v (a f) -> (v a) f", f=FC)
            ftemb_sb = const.tile([4, FC], f32)
            nc.gpsimd.indirect_dma_start(
                out=ftemb_sb[:], out_offset=None, in_=emb8,
                in_offset=bass.IndirectOffsetOnAxis(ap=idft_sb[:, 0:1], axis=0),
            )

            # ---- phase 1: V = att @ W (one 8x256 accumulation) ---------------
            vsb = const.tile([2 * H, EC], bf16)
            vT_sb = [const.tile([128, 2 * H], bf16, name=f"vT{i}", tag=f"vT{i}")
                     for i in range(2)]
            a_bf = const.tile([128, 68], bf16)
            with tc.tile_pool(name="pp1", bufs=1, space="PSUM") as pp1, \
                 tc.tile_pool(name="pp1t", bufs=2, space="PSUM") as pp1t:
                ps_v2 = pp1.tile([2 * H, EC], f32)
                for t in range(NT):
                    nc.tensor.matmul(out=ps_v2[:], lhsT=attst_sb[:, t, :],
                                     rhs=wn_sb[:, t, :],
                                     start=(t == 0), stop=(t == NT - 1))
                nc.vector.tensor_copy(out=vsb[:], in_=ps_v2[:])
                for half in range(2):
                    ps_t = pp1t.tile([128, 2 * H], bf16, tag="pst")
                    nc.tensor.transpose(out=ps_t[:],
                                        in_=vsb[:, half * 128:(half + 1) * 128],
                                        identity=ident_sb[0:2 * H, 0:2 * H])
                    nc.vector.tensor_copy(out=vT_sb[half][:], in_=ps_t[:])
                # a[j, c] partial, j-major: ps_a[128, u*8 + c]
                ps_a = pp1.tile([128, 128], f32)
                for jc in range(NU):
                    for half in range(2):
                        nc.tensor.matmul(
                            out=ps_a[:, jc * 8:(jc + 1) * 8],
                            lhsT=nft_sb[half][:, jc * 128:(jc + 1) * 128],
                            rhs=vT_sb[half][:],
                            start=(half == 0), stop=(half == 1))
                # pack: src logits [128, 16*4] + a_dst partials of the last
                # j-block in cols 64:68 (only row 127 = j 2047 is used)
                psa_v = ps_a[:].rearrange("p (u c) -> p u c", c=8)
                abf_v = a_bf[:, 0:64].rearrange("p (u c) -> p u c", c=4)
                nc.vector.tensor_copy(out=abf_v, in_=psa_v[:, :, 0:H])
                nc.vector.tensor_copy(out=a_bf[:, 64:68],
                                      in_=ps_a[:, 15 * 8 + H:15 * 8 + 2 * H])


            # ---- AllReduce #1: attention logits (bf16, Shared out) -----------
            ar1_in = dram.tile([128, 68], bf16, tag="ar1i")
            ar1_out = dram.tile([128, 68], bf16, tag="ar1o", addr_space="Shared")
            nc.scalar.dma_start(ar1_in[:], a_bf[:])
            nc.gpsimd.collective_compute(
                "AllReduce", AT.add, replica_groups=RG,
                ins=[ar1_in[:].opt()], outs=[ar1_out[:].opt()])

            # ---- softmax weights (replicated) --------------------------------
            a_sb = const.tile([128, 68], bf16)
            nc.scalar.dma_start(a_sb[:], ar1_out[:])
            dst1_sb = const.tile([1, H], bf16)
            nc.scalar.dma_start(dst1_sb[:], ar1_out[127:128, 64:68])
            wu_exp = const.tile([128, NU, H], bf16)
            with tc.tile_pool(name="ppd", bufs=1, space="PSUM") as ppd:
                ps_dst = ppd.tile([128, H], f32)
                nc.tensor.matmul(out=ps_dst[:], lhsT=ones1b_sb[:], rhs=dst1_sb[:],
                                 start=True, stop=True)
                dstb_sb = const.tile([128, H], f32)
                nc.vector.tensor_copy(out=dstb_sb[:], in_=ps_dst[:])
            dstb_b = bass.AP(tensor=dstb_sb[:].tensor, offset=dstb_sb[:].offset,
                             ap=[dstb_sb[:].ap[0], [0, NU], [1, H]])
            a_srcv = a_sb[:, 0:64].rearrange("p (u c) -> p u c", c=4)
            l_sb = const.tile([128, NU, H], f32)
            nc.vector.tensor_tensor(out=l_sb[:], in0=a_srcv, in1=dstb_b, op=AT.add)
            l2_sb = const.tile([128, NU, H], f32)
            nc.vector.tensor_scalar_mul(l2_sb[:], l_sb[:], 0.2)
            nc.vector.tensor_tensor(out=l_sb[:], in0=l_sb[:], in1=l2_sb[:], op=AT.max)
            nc.scalar.activation(out=wu_exp[:], in_=l_sb[:],
                                 func=mybir.ActivationFunctionType.Exp)
            dummy2_sb = const.tile([1, 1], f32)
            nc.vector.memset(dummy2_sb[:], 1.0)
            nc.scalar.activation(out=dummy2_sb[:], in_=dummy2_sb[:],
                                 func=mybir.ActivationFunctionType.Sqrt)

            # ---- agg = attnU^T @ [nf | 1]; normalize; broadcast --------------
            with tc.tile_pool(name="ppg", bufs=1, space="PSUM") as ppg, \
                 tc.tile_pool(name="ppab", bufs=2, space="PSUM") as ppab:
                ps_agg = ppg.tile([H, EC + 1], f32)
                for u in range(NU):
                    nc.tensor.matmul(
                        out=ps_agg[:], lhsT=wu_exp[:, u, :], rhs=nf_sb[:, u, :],
                        start=(u == 0), stop=(u == NU - 1))
                rz_sb = const.tile([H, 1], f32)
                nc.vector.reciprocal(out=rz_sb[:], in_=ps_agg[:, EC:EC + 1])
                nc.vector.tensor_scalar_mul(rz_sb[:], rz_sb[:], 1.0 / H)
                aggn_sb = const.tile([H, EC], bf16)
                nc.vector.tensor_scalar_mul(aggn_sb[:], ps_agg[:, 0:EC], rz_sb[:])
                aggT_sb = [const.tile([128, H], bf16, name=f"aggT{i}", tag=f"aggT{i}")
                           for i in range(2)]
                for half in range(2):
                    ps_gt = ppab.tile([128, H], bf16, tag="psgt")
                    nc.tensor.transpose(out=ps_gt[:],
                                        in_=aggn_sb[:, half * 128:(half + 1) * 128],
                                        identity=ident_sb[0:H, 0:H])
                    nc.vector.tensor_copy(out=aggT_sb[half][:], in_=ps_gt[:])

            # ---- out[-1] partial, free-major [1, 2048] + ridden sum ----------
            outp_row = const.tile([1, AR2W], f32)
            nc.vector.memset(outp_row[:, D + 1:], 0.0)
            s4_sb = const.tile([1, 4], f32)
            with tc.tile_pool(name="ppo", bufs=1, space="PSUM") as ppo:
                for q in range(4):
                    ps_o = ppo.tile([1, 512], f32, name=f"pso{q}", tag=f"pso{q}")
                    for h in range(H):
                        for half in range(2):
                            nc.tensor.matmul(
                                out=ps_o[:],
                                lhsT=aggT_sb[half][:, h:h + 1],
                                rhs=wt_sb[half][:, h * D + q * 512:h * D + (q + 1) * 512],
                                start=(h == 0 and half == 0),
                                stop=(h == H - 1 and half == 1))
                    nc.vector.tensor_tensor(
                        out=outp_row[:, q * 512:(q + 1) * 512], in0=ps_o[:],
                        in1=gbrow_sb[:, q * 512:(q + 1) * 512], op=AT.add)
                    nc.vector.reduce_sum(out=s4_sb[:, q:q + 1],
                                         in_=outp_row[:, q * 512:(q + 1) * 512],
                                         axis=mybir.AxisListType.X)
            nc.vector.reduce_sum(out=outp_row[:, D:D + 1], in_=s4_sb[:],
                                 axis=mybir.AxisListType.X)

            ar2_in = dram.tile([1, AR2W], f32, tag="ar2i")
            ar2_out = dram.tile([1, AR2W], f32, tag="ar2o", addr_space="Shared")
            nc.scalar.dma_start(ar2_in[:], outp_row[:])
            nc.gpsimd.collective_compute(
                "AllReduce", AT.add, replica_groups=RG,
                ins=[ar2_in[:].opt()], outs=[ar2_out[:].opt()])

            # ---- LayerNorm (replicated), single partition-major readback -----
            lnx_sb = const.tile([128, 17], f32)
            ar2ap = ar2_out[:]
            ln_src = bass.AP(tensor=ar2ap.tensor, offset=ar2ap.offset,
                             ap=[[1, 128], [128, 17]])
            nc.scalar.dma_start(lnx_sb[:], ln_src)
            scr_sb = const.tile([128, NU], f32)
            sq_p = const.tile([128, 1], f32)
            nc.scalar.activation(out=scr_sb[:], in_=lnx_sb[:, 0:16],
                                 func=mybir.ActivationFunctionType.Square,
                                 accum_out=sq_p[:])
            stats_sb = const.tile([1, 2], f32)
            nc.vector.tensor_scalar_mul(stats_sb[:, 0:1], lnx_sb[0:1, 16:17], 1.0 / D)
            m2_sb = const.tile([1, 1], f32)
            var_sb = const.tile([1, 1], f32)
            eps_sb = const.tile([1, 1], f32)
            nc.vector.memset(eps_sb[:], 1e-5)
            mem_sb = const.tile([128, NU], f32)
            memb_sb = const.tile([128, NU], bf16)
            with tc.tile_pool(name="ppl", bufs=2, space="PSUM") as ppl:
                ps_s2 = ppl.tile([1, 1], f32, tag="s2")
                nc.tensor.matmul(out=ps_s2[:], lhsT=onescol_sb[:], rhs=sq_p[:],
                                 start=True, stop=True)
                nc.vector.tensor_scalar_mul(m2_sb[:], ps_s2[:], 1.0 / D)
                nc.vector.tensor_tensor(out=var_sb[:], in0=stats_sb[:, 0:1],
                                        in1=stats_sb[:, 0:1], op=AT.mult)
                nc.vector.tensor_tensor(out=var_sb[:], in0=m2_sb[:], in1=var_sb[:],
                                        op=AT.subtract)
                nc.scalar.activation(out=var_sb[:], in_=var_sb[:],
                                     func=mybir.ActivationFunctionType.Sqrt,
                                     bias=eps_sb[:], scale=1.0)
                nc.vector.reciprocal(out=stats_sb[:, 1:2], in_=var_sb[:])
                ps_b = ppl.tile([128, 2], f32, tag="bc")
                nc.tensor.matmul(out=ps_b[:], lhsT=ones1_sb[:], rhs=stats_sb[:],
                                 start=True, stop=True)
                bc_sb = const.tile([128, 2], f32)
                nc.vector.tensor_copy(out=bc_sb[:], in_=ps_b[:])
            nc.vector.tensor_scalar(out=mem_sb[:], in0=lnx_sb[:, 0:16],
                                    scalar1=bc_sb[:, 0:1], scalar2=bc_sb[:, 1:2],
                                    op0=AT.subtract, op1=AT.mult)
            nc.vector.tensor_mul(mem_sb[:], mem_sb[:], gamma_sb[:])
            nc.vector.tensor_add(mem_sb[:], mem_sb[:], beta_sb[:])
            nc.vector.tensor_copy(out=memb_sb[:], in_=mem_sb[:])

            # ---- proj + LoRA offset chunk; add into first-token pieces -------
            with tc.tile_pool(name="ppp", bufs=1, space="PSUM") as ppp:
                ps_lt = ppp.tile([R, 1], f32, tag="lt")
                for u in range(NU):
                    nc.tensor.matmul(out=ps_lt[:], lhsT=lat_sb[:, u, :],
                                     rhs=memb_sb[:, u:u + 1],
                                     start=(u == 0), stop=(u == NU - 1))
                lt2_sb = const.tile([R, 1], bf16)
                nc.vector.tensor_scalar_mul(lt2_sb[:], ps_lt[:], 2.0)  # alpha/r
                ps_pj = ppp.tile([1, FC], f32, tag="pj")
                for u in range(NU):
                    nc.tensor.matmul(
                        out=ps_pj[:], lhsT=memb_sb[:, u:u + 1],
                        rhs=proj_sb[:, u, :], start=(u == 0), stop=False)
                nc.tensor.matmul(out=ps_pj[:], lhsT=lt2_sb[:], rhs=lbt_sb[:],
                                 start=False, stop=True)
                off_sb = const.tile([1, FC], f32)
                nc.vector.tensor_add(off_sb[:], ps_pj[:], projb_sb[:])
            ftout_sb = const.tile([4, FC], f32)
            with tc.tile_pool(name="ppf", bufs=1, space="PSUM") as ppf:
                ps_ft = ppf.tile([4, FC], f32, tag="ft")
                nc.tensor.matmul(out=ps_ft[:], lhsT=ones1_sb[0:1, 0:4],
                                 rhs=off_sb[:], start=True, stop=True)
                nc.vector.tensor_tensor(out=ftout_sb[:], in0=ftemb_sb[:],
                                        in1=ps_ft[:], op=AT.add)
            nc.sync.dma_start(ft_out[:], ftout_sb[:])

    nc.compile()
    return nc


def _prep_inputs(inputs):
    import ml_dtypes
    bf16 = ml_dtypes.bfloat16
    fp8 = ml_dtypes.float8_e4m3

    nf = np.asarray(inputs["node_features"], dtype=np.float32)
    ids = np.asarray(inputs["input_ids"], dtype=np.int32)
    ids_flat = ids.reshape(-1)
    gw = np.asarray(inputs["gat_w"], dtype=np.float32)
    att_src = np.asarray(inputs["att_src"], dtype=np.float32)
    att_dst = np.asarray(inputs["att_dst"], dtype=np.float32)
    gbias = np.asarray(inputs["gat_bias"], dtype=np.float32)
    gamma = np.asarray(inputs["ln_gamma"], dtype=np.float32)
    beta = np.asarray(inputs["ln_beta"], dtype=np.float32)
    pw = np.asarray(inputs["proj_w"], dtype=np.float32)
    pb = np.asarray(inputs["proj_b"], dtype=np.float32)
    la = np.asarray(inputs["lora_a"], dtype=np.float32)
    lb = np.asarray(inputs["lora_b"], dtype=np.float32)
    emb = np.ascontiguousarray(np.asarray(inputs["embed"], dtype=np.float32))

    def chunked(vec, parts=2):  # [parts*128] -> [128, parts]
        return np.ascontiguousarray(vec.reshape(parts, 128).T)

    def pre3(m, inner):  # [NU*128, inner] -> [128, NU, inner]
        return np.ascontiguousarray(
            m.reshape(NU, 128, inner).transpose(1, 0, 2).astype(bf16))

    att_strips = np.zeros((NT, 128, 2 * H), dtype=np.float32)
    for t in range(NT):
        h, u = t // NU, t % NU
        att_strips[t, :, h] = att_src[h, u * 128:(u + 1) * 128]
        att_strips[t, :, H + h] = att_dst[h, u * 128:(u + 1) * 128]
    att_st = np.ascontiguousarray(
        att_strips.transpose(1, 0, 2).astype(bf16))  # [128, NT, 2H]
    lora_a_pre = pre3(la.T, R)
    gamma_r = chunked(gamma, NU)
    beta_r = chunked(beta, NU)

    in_maps = []
    for c in range(NCORES):
        ech = slice(c * EC, (c + 1) * EC)
        fch = slice(c * FC, (c + 1) * FC)
        w_sl = gw[:, ech]
        nf_sl = nf[:, ech]
        m = {
            "w_nat": np.ascontiguousarray(w_sl.reshape(NT, 128, EC).transpose(1, 0, 2).astype(bf16)),
            "w_tr": np.ascontiguousarray(w_sl.T.astype(bf16)),
            "nf_pre": pre3(nf_sl, EC),
            "nf_tr": np.ascontiguousarray(nf_sl.T.astype(bf16)),
            "att_st": att_st,
            "proj_pre": pre3(pw[fch, :].T, FC),
            "projb_r": np.ascontiguousarray(pb[fch].reshape(1, FC)),
            "lora_a_pre": lora_a_pre,
            "lora_bt": np.ascontiguousarray(lb[fch, :].T.astype(bf16)),
            "gbias_row": (gbias.reshape(1, D).astype(np.float32)
                          if c == 0 else np.zeros((1, D), dtype=np.float32)),
            "gamma_r": gamma_r,
            "beta_r": beta_r,
            "ids_r": np.ascontiguousarray(
                ids_flat[c * ROWS:(c + 1) * ROWS].reshape(NG, 128).T),
            "ids_ft": np.ascontiguousarray(
                (ids[:, 0].astype(np.int64) * NCORES + c).astype(np.int32)
                .reshape(4, 1)),
            "embed": emb,
        }
        in_maps.append(m)
    return in_maps


def kernel(**inputs):
    _install_ntff_shim()
    from concourse.bass_utils import run_bass_kernel_spmd

    if "nc" not in _CACHE:
        _CACHE["nc"] = _build()
    nc = _CACHE["nc"]

    in_maps = _prep_inputs(inputs)
    trace = bool(int(os.environ.get("KERNEL_TRACE", "0")))
    res = run_bass_kernel_spmd(nc, in_maps, core_ids=list(range(NCORES)),
                               trace=trace)
    if trace:
        _CACHE["last_result"] = res
        print(f"HW exec time: {res.exec_time_ns} ns", flush=True)

    out = np.concatenate([res.results[c]["out_sl"] for c in range(NCORES)], axis=0)
    out = out.reshape(B, S, D)
    ft = np.concatenate([res.results[c]["ft_out"] for c in range(NCORES)], axis=1)
    out[:, 0, :] = ft
    return out


# revision 26
# speedup vs baseline: 1.3554x; 1.3554x over previous
"""Trainium2 Bass kernel for nn_MemoryAugmentedModel (gnn_message_passing).

Math: the reference only consumes row N-1 of the GAT output, so the dense
[N,N,H] attention collapses:
  out[-1] = (1/H) * sum_h gat_w_h @ (softmax_j(lrelu(a_dst[-1,h]+a_src[j,h])) @ nf) + gat_bias
with a_src = nf @ V_src^T, V_src[h] = att_src[h] @ gat_w_h  (same for dst).
Then LayerNorm -> proj/LoRA offset -> embedding gather with offset added to
each sequence's first token.

Sharding (8 cores): gat_w / node_features split by input-feature columns
(e-chunks of 256) -> partial attention logits (AllReduce #1, bf16 [128,68])
-> replicated softmax -> per-core agg over its e-chunk -> partial out[-1]
(AllReduce #2, f32 [1,2176] with ridden row-sum) -> replicated LayerNorm ->
proj/LoRA offset chunk [1,256] added directly to column-gathered first-token
embedding pieces [4,256] (aux output; host stitches) -- no AllGather needed.
Each core also gathers 1024 of the 8192 output embedding rows.

Schedule: a warmup AllReduce at t=0 absorbs the first-collective channel
init. Chain-critical inputs (att, w_nat, nf) load on the sync queue (then
the sync queue takes the gather writeback); bulk post-AR1 weights (w_tr,
proj, lora) plus the post-collective readbacks and activations run on the
scalar queue; the embedding gather is gated behind AR1's trigger in gpsimd
program order so it never starves the serial chain.
"""

import os
import sys
import types

import numpy as np

NCORES = 8
N = 2048
D = 2048
H = 4
R = 32
V = 32000
B = 4
S = 2048

EC = D // NCORES          # 256: e-columns (input features) per core
FC = D // NCORES          # 256: offset cols per core
ROWS = (B * S) // NCORES  # 1024: output embedding rows per core
NG = ROWS // 128          # 8 gather groups per core
NU = D // 128             # 16: 128-row chunks of a length-D axis
NT = (H * D) // 128       # 64: 128-row strips of gat_w
AR2W = 17 * 128           # 2176: AR2 payload (2048 row + sum + pad)

_CACHE = {}


def _install_ntff_shim():
    """Register the axon NTFF profile hook missing from this image's antenv."""
    if "antenv.axon_hooks" in sys.modules:
        return
    try:
        import antenv
        from trn_agent_boot.trn_boot import _ntff_profile_via_ctypes
    except Exception:
        return
    mod = types.ModuleType("antenv.axon_hooks")
    mod._hook = None
    mod.set_axon_ntff_profile_hook = lambda h: setattr(mod, "_hook", h)
    mod.get_axon_ntff_profile_hook = lambda: mod._hook
    sys.modules["antenv.axon_hooks"] = mod
    antenv.axon_hooks = mod
    try:
        mod.set_axon_ntff_profile_hook(
            _ntff_profile_via_ctypes("/opt/axon/libaxon_pjrt.so")
        )
    except Exception:
        pass


def _build():
    import concourse.bacc as bacc
    import concourse.bass as bass
    import concourse.tile as tile
    from concourse import mybir

    f32 = mybir.dt.float32
    bf16 = mybir.dt.bfloat16
    fp8 = mybir.dt.float8e4
    i32 = mybir.dt.int32
    RG = [list(range(NCORES))]
    AT = mybir.AluOpType

    nc = bacc.Bacc("TRN2", target_bir_lowering=False, debug=False,
                   num_devices=NCORES)

    din = lambda name, shape, dt: nc.dram_tensor(name, shape, dt, kind="ExternalInput").ap()
    w_nat = din("w_nat", [128, NT, EC], bf16)
    w_tr = din("w_tr", [2 * 128, H * D], bf16)
    att_st = din("att_st", [128, NT, 2 * H], bf16)   # zero-padded per strip
    nf_pre = din("nf_pre", [128, NU, EC], bf16)
    nf_tr = din("nf_tr", [2 * 128, N], bf16)
    proj_pre = din("proj_pre", [128, NU, FC], bf16)
    projb_r = din("projb_r", [1, FC], f32)
    lora_a_pre = din("lora_a_pre", [128, NU, R], bf16)
    lora_bt = din("lora_bt", [R, FC], bf16)
    gbias_row = din("gbias_row", [1, D], f32)
    gamma_r = din("gamma_r", [128, NU], f32)
    beta_r = din("beta_r", [128, NU], f32)
    ids_r = din("ids_r", [128, NG], i32)
    ids_ft = din("ids_ft", [4, 1], i32)   # first-token ids, pre-scaled *8+core
    embed = din("embed", [V, D], bf16)

    out_sl = nc.dram_tensor("out_sl", [ROWS, D], bf16, kind="ExternalOutput").ap()
    ft_out = nc.dram_tensor("ft_out", [4, FC], f32, kind="ExternalOutput").ap()

    with tile.TileContext(nc) as tc:
        import contextlib
        ctx = contextlib.ExitStack()
        with ctx:
            const = ctx.enter_context(tc.tile_pool(name="const", bufs=1))
            embp = ctx.enter_context(tc.tile_pool(name="embp", bufs=NG))
            dram = ctx.enter_context(tc.tile_pool(name="dram", bufs=1, space="DRAM"))

            ids_st = const.tile([128, NG], i32)
            nc.gpsimd.dma_start(ids_st[:], ids_r[:])
            idft_sb = const.tile([4, 1], i32)
            nc.gpsimd.dma_start(idft_sb[:], ids_ft[:])

            # ---- chain-critical inputs split across both HW queues -----------
            attst_sb = const.tile([128, NT, 2 * H], bf16)
            nc.sync.dma_start(attst_sb[:], att_st[:])
            wn_sb = const.tile([128, NT, EC], bf16)
            for ch in range(4):
                nc.sync.dma_start(wn_sb[:, ch * 16:(ch + 1) * 16, :],
                                  w_nat[:, ch * 16:(ch + 1) * 16, :])
            nft_sb = []
            for half in range(2):
                t = const.tile([128, N], bf16, name=f"nft{half}", tag=f"nft{half}")
                nc.scalar.dma_start(t[:], nf_tr[half * 128:(half + 1) * 128, :])
                nft_sb.append(t)
            nf_sb = const.tile([128, NU, EC + 1], bf16)
            nc.scalar.dma_start(nf_sb[:, :, 0:EC], nf_pre[:])
            nc.vector.memset(nf_sb[:, :, EC:EC + 1], 1.0)

            # ---- bulk post-AR1 weights behind the chain inputs on sync -------
            wt_sb = []
            for half in range(2):
                t = const.tile([128, H * D], bf16, name=f"wt{half}", tag=f"wt{half}")
                nc.sync.dma_start(t[:], w_tr[half * 128:(half + 1) * 128, :])
                wt_sb.append(t)
            proj_sb = const.tile([128, NU, FC], bf16)
            nc.sync.dma_start(proj_sb[:], proj_pre[:])
            projb_sb = const.tile([1, FC], f32)
            nc.sync.dma_start(projb_sb[:], projb_r[:])
            lat_sb = const.tile([128, NU, R], bf16)
            nc.sync.dma_start(lat_sb[:], lora_a_pre[:])
            lbt_sb = const.tile([R, FC], bf16)
            nc.sync.dma_start(lbt_sb[:], lora_bt[:])
            gbrow_sb = const.tile([1, D], f32)
            nc.sync.dma_start(gbrow_sb[:], gbias_row[:])
            gamma_sb = const.tile([128, NU], f32)
            nc.sync.dma_start(gamma_sb[:], gamma_r[:])
            beta_sb = const.tile([128, NU], f32)
            nc.sync.dma_start(beta_sb[:], beta_r[:])
            ident_sb = const.tile([128, 128], bf16)
            from concourse.masks import make_identity
            make_identity(nc, ident_sb[:])
            ones1_sb = const.tile([1, 128], f32)
            nc.vector.memset(ones1_sb[:], 1.0)
            ones1b_sb = const.tile([1, 128], bf16)
            nc.vector.memset(ones1b_sb[:], 1.0)
            onescol_sb = const.tile([128, 1], f32)
            nc.vector.memset(onescol_sb[:], 1.0)
            # preload the activation table so exp doesn't pay for it mid-chain
            dummy_sb = const.tile([1, 1], f32)
            nc.vector.memset(dummy_sb[:], 0.0)
            nc.scalar.activation(out=dummy_sb[:], in_=dummy_sb[:],
                                 func=mybir.ActivationFunctionType.Exp)

            # ---- phase 1: V = att @ W (one 8x256 accumulation) ---------------
            vsb = const.tile([2 * H, EC], bf16)
            vT_sb = [const.tile([128, 2 * H], bf16, name=f"vT{i}", tag=f"vT{i}")
                     for i in range(2)]
            a_bf = const.tile([128, 68], bf16)
            with tc.tile_pool(name="pp1", bufs=1, space="PSUM") as pp1, \
                 tc.tile_pool(name="pp1t", bufs=2, space="PSUM") as pp1t:
                ps_v2 = pp1.tile([2 * H, EC], f32)
                for t in range(NT):
                    nc.tensor.matmul(out=ps_v2[:], lhsT=attst_sb[:, t, :],
                                     rhs=wn_sb[:, t, :],
                                     start=(t == 0), stop=(t == NT - 1))
                nc.vector.tensor_copy(out=vsb[:], in_=ps_v2[:])
                for half in range(2):
                    ps_t = pp1t.tile([128, 2 * H], bf16, tag="pst")
                    nc.tensor.transpose(out=ps_t[:],
                                        in_=vsb[:, half * 128:(half + 1) * 128],
                                        identity=ident_sb[0:2 * H, 0:2 * H])
                    nc.vector.tensor_copy(out=vT_sb[half][:], in_=ps_t[:])
                # a[j, c] partial, j-major: ps_a[128, u*8 + c]
                ps_a = pp1.tile([128, 128], f32)
                for jc in range(NU):
                    for half in range(2):
                        nc.tensor.matmul(
                            out=ps_a[:, jc * 8:(jc + 1) * 8],
                            lhsT=nft_sb[half][:, jc * 128:(jc + 1) * 128],
                            rhs=vT_sb[half][:],
                            start=(half == 0), stop=(half == 1))
                # pack: src logits [128, 16*4] + a_dst partials of the last
                # j-block in cols 64:68 (only row 127 = j 2047 is used)
                psa_v = ps_a[:].rearrange("p (u c) -> p u c", c=8)
                abf_v = a_bf[:, 0:64].rearrange("p (u c) -> p u c", c=4)
                nc.vector.tensor_copy(out=abf_v, in_=psa_v[:, :, 0:H])
                nc.vector.tensor_copy(out=a_bf[:, 64:68],
                                      in_=ps_a[:, 15 * 8 + H:15 * 8 + 2 * H])


            # ---- AllReduce #1: attention logits (bf16, Shared out) -----------
            ar1_in = dram.tile([128, 68], bf16, tag="ar1i")
            ar1_out = dram.tile([128, 68], bf16, tag="ar1o", addr_space="Shared")
            nc.scalar.dma_start(ar1_in[:], a_bf[:])
            nc.gpsimd.collective_compute(
                "AllReduce", AT.add, replica_groups=RG,
                ins=[ar1_in[:].opt()], outs=[ar1_out[:].opt()])

            # ---- embedding gather (bf16 table; host upcasts the output) ------
            # gens sit behind AR1's trigger stall on gpsimd and are dep-gated
            # on a_bf via ids_sb, keeping the CC bootstrap window quiet
            gatef_sb = const.tile([1, 1], f32)
            nc.vector.tensor_scalar_mul(gatef_sb[:], a_bf[0:1, 0:1], 0.0)
            gcol_sb = const.tile([128, 1], i32)
            with tc.tile_pool(name="ppz", bufs=1, space="PSUM") as ppz:
                ps_z = ppz.tile([128, 1], f32, tag="z")
                nc.tensor.matmul(out=ps_z[:], lhsT=ones1_sb[:], rhs=gatef_sb[:],
                                 start=True, stop=True)
                nc.vector.tensor_copy(out=gcol_sb[:], in_=ps_z[:])
            ids_sb = const.tile([128, NG], i32)
            gz_b = bass.AP(tensor=gcol_sb[:].tensor, offset=gcol_sb[:].offset,
                           ap=[gcol_sb[:].ap[0], [0, NG]])
            nc.vector.tensor_tensor(out=ids_sb[:], in0=ids_st[:], in1=gz_b,
                                    op=AT.add)
            emb_tiles = []
            for g in range(NG):
                et = embp.tile([128, D], bf16, name=f"emb{g}", tag="emb")
                nc.gpsimd.indirect_dma_start(
                    out=et[:], out_offset=None, in_=embed[:, :],
                    in_offset=bass.IndirectOffsetOnAxis(ap=ids_sb[:, g:g + 1], axis=0),
                )
                emb_tiles.append(et)
                nc.sync.dma_start(out_sl[g * 128:(g + 1) * 128, :], et[:])
            # first-token embedding column pieces [4, FC] for this core's chunk
            # (ids pre-scaled host-side: idx = id*8 + core, over a [V*8, FC] view)
            emb8 = embed[:, :].rearrange("v (a f) -> (v a) f", f=FC)
            ftemb_sb = const.tile([4, FC], bf16)
            nc.gpsimd.indirect_dma_start(
                out=ftemb_sb[:], out_offset=None, in_=emb8,
                in_offset=bass.IndirectOffsetOnAxis(ap=idft_sb[:, 0:1], axis=0),
            )

            # ---- softmax weights (replicated) --------------------------------
            a_sb = const.tile([128, 68], bf16)
            nc.scalar.dma_start(a_sb[:], ar1_out[:])
            dst1_sb = const.tile([1, H], bf16)
            nc.scalar.dma_start(dst1_sb[:], ar1_out[127:128, 64:68])
            wu_exp = const.tile([128, NU, H], bf16)
            with tc.tile_pool(name="ppd", bufs=1, space="PSUM") as ppd:
                ps_dst = ppd.tile([128, H], f32)
                nc.tensor.matmul(out=ps_dst[:], lhsT=ones1b_sb[:], rhs=dst1_sb[:],
                                 start=True, stop=True)
                dstb_sb = const.tile([128, H], f32)
                nc.vector.tensor_copy(out=dstb_sb[:], in_=ps_dst[:])
            dstb_b = bass.AP(tensor=dstb_sb[:].tensor, offset=dstb_sb[:].offset,
                             ap=[dstb_sb[:].ap[0], [0, NU], [1, H]])
            a_srcv = a_sb[:, 0:64].rearrange("p (u c) -> p u c", c=4)
            l_sb = const.tile([128, NU, H], f32)
            nc.vector.tensor_tensor(out=l_sb[:], in0=a_srcv, in1=dstb_b, op=AT.add)
            l2_sb = const.tile([128, NU, H], f32)
            nc.vector.tensor_scalar_mul(l2_sb[:], l_sb[:], 0.2)
            nc.vector.tensor_tensor(out=l_sb[:], in0=l_sb[:], in1=l2_sb[:], op=AT.max)
            nc.scalar.activation(out=wu_exp[:], in_=l_sb[:],
                                 func=mybir.ActivationFunctionType.Exp)
            dummy2_sb = const.tile([1, 1], f32)
            nc.vector.memset(dummy2_sb[:], 1.0)
            nc.scalar.activation(out=dummy2_sb[:], in_=dummy2_sb[:],
                                 func=mybir.ActivationFunctionType.Sqrt)

            # ---- agg = attnU^T @ [nf | 1]; normalize; broadcast --------------
            with tc.tile_pool(name="ppg", bufs=1, space="PSUM") as ppg, \
                 tc.tile_pool(name="ppab", bufs=2, space="PSUM") as ppab:
                ps_agg = ppg.tile([H, EC + 1], f32)
                for u in range(NU):
                    nc.tensor.matmul(
                        out=ps_agg[:], lhsT=wu_exp[:, u, :], rhs=nf_sb[:, u, :],
                        start=(u == 0), stop=(u == NU - 1))
                rz_sb = const.tile([H, 1], f32)
                nc.vector.reciprocal(out=rz_sb[:], in_=ps_agg[:, EC:EC + 1])
                nc.vector.tensor_scalar_mul(rz_sb[:], rz_sb[:], 1.0 / H)
                aggn_sb = const.tile([H, EC], bf16)
                nc.vector.tensor_scalar_mul(aggn_sb[:], ps_agg[:, 0:EC], rz_sb[:])
                aggT_sb = [const.tile([128, H], bf16, name=f"aggT{i}", tag=f"aggT{i}")
                           for i in range(2)]
                for half in range(2):
                    ps_gt = ppab.tile([128, H], bf16, tag="psgt")
                    nc.tensor.transpose(out=ps_gt[:],
                                        in_=aggn_sb[:, half * 128:(half + 1) * 128],
                                        identity=ident_sb[0:H, 0:H])
                    nc.vector.tensor_copy(out=aggT_sb[half][:], in_=ps_gt[:])

            # ---- out[-1] partial, free-major [1, 2048] + ridden sum ----------
            outp_row = const.tile([1, AR2W], f32)
            nc.vector.memset(outp_row[:, D + 1:], 0.0)
            s4_sb = const.tile([1, 4], f32)
            with tc.tile_pool(name="ppo", bufs=1, space="PSUM") as ppo:
                for q in range(4):
                    ps_o = ppo.tile([1, 512], f32, name=f"pso{q}", tag=f"pso{q}")
                    for h in range(H):
                        for half in range(2):
                            nc.tensor.matmul(
                                out=ps_o[:],
                                lhsT=aggT_sb[half][:, h:h + 1],
                                rhs=wt_sb[half][:, h * D + q * 512:h * D + (q + 1) * 512],
                                start=(h == 0 and half == 0),
                                stop=(h == H - 1 and half == 1))
                    nc.vector.tensor_tensor(
                        out=outp_row[:, q * 512:(q + 1) * 512], in0=ps_o[:],
                        in1=gbrow_sb[:, q * 512:(q + 1) * 512], op=AT.add)
                    nc.vector.reduce_sum(out=s4_sb[:, q:q + 1],
                                         in_=outp_row[:, q * 512:(q + 1) * 512],
                                         axis=mybir.AxisListType.X)
            nc.vector.reduce_sum(out=outp_row[:, D:D + 1], in_=s4_sb[:],
                                 axis=mybir.AxisListType.X)

            ar2_in = dram.tile([1, AR2W], f32, tag="ar2i")
            ar2_out = dram.tile([1, AR2W], f32, tag="ar2o", addr_space="Shared")
            nc.scalar.dma_start(ar2_in[:], outp_row[:])
            nc.gpsimd.collective_compute(
                "AllReduce", AT.add, replica_groups=RG,
                ins=[ar2_in[:].opt()], outs=[ar2_out[:].opt()])

            # ---- LayerNorm (replicated), single partition-major readback -----
            lnx_sb = const.tile([128, 17], f32)
            ar2ap = ar2_out[:]
            ln_src = bass.AP(tensor=ar2ap.tensor, offset=ar2ap.offset,
                             ap=[[1, 128], [128, 17]])
            nc.scalar.dma_start(lnx_sb[:], ln_src)
            scr_sb = const.tile([128, NU], f32)
            sq_p = const.tile([128, 1], f32)
            nc.scalar.activation(out=scr_sb[:], in_=lnx_sb[:, 0:16],
                                 func=mybir.ActivationFunctionType.Square,
                                 accum_out=sq_p[:])
            stats_sb = const.tile([1, 2], f32)
            nc.vector.tensor_scalar_mul(stats_sb[:, 0:1], lnx_sb[0:1, 16:17], 1.0 / D)
            m2_sb = const.tile([1, 1], f32)
            var_sb = const.tile([1, 1], f32)
            eps_sb = const.tile([1, 1], f32)
            nc.vector.memset(eps_sb[:], 1e-5)
            mem_sb = const.tile([128, NU], f32)
            memb_sb = const.tile([128, NU], bf16)
            with tc.tile_pool(name="ppl", bufs=2, space="PSUM") as ppl:
                ps_s2 = ppl.tile([1, 1], f32, tag="s2")
                nc.tensor.matmul(out=ps_s2[:], lhsT=onescol_sb[:], rhs=sq_p[:],
                                 start=True, stop=True)
                nc.vector.tensor_scalar_mul(m2_sb[:], ps_s2[:], 1.0 / D)
                nc.vector.tensor_tensor(out=var_sb[:], in0=stats_sb[:, 0:1],
                                        in1=stats_sb[:, 0:1], op=AT.mult)
                nc.vector.tensor_tensor(out=var_sb[:], in0=m2_sb[:], in1=var_sb[:],
                                        op=AT.subtract)
                nc.scalar.activation(out=var_sb[:], in_=var_sb[:],
                                     func=mybir.ActivationFunctionType.Sqrt,
                                     bias=eps_sb[:], scale=1.0)
                nc.vector.reciprocal(out=stats_sb[:, 1:2], in_=var_sb[:])
                ps_b = ppl.tile([128, 2], f32, tag="bc")
                nc.tensor.matmul(out=ps_b[:], lhsT=ones1_sb[:], rhs=stats_sb[:],
                                 start=True, stop=True)
                bc_sb = const.tile([128, 2], f32)
                nc.vector.tensor_copy(out=bc_sb[:], in_=ps_b[:])
            nc.vector.tensor_scalar(out=mem_sb[:], in0=lnx_sb[:, 0:16],
                                    scalar1=bc_sb[:, 0:1], scalar2=bc_sb[:, 1:2],
                                    op0=AT.subtract, op1=AT.mult)
            nc.vector.tensor_mul(mem_sb[:], mem_sb[:], gamma_sb[:])
            nc.vector.tensor_add(mem_sb[:], mem_sb[:], beta_sb[:])
            nc.vector.tensor_copy(out=memb_sb[:], in_=mem_sb[:])

            # ---- proj + LoRA offset chunk; add into first-token pieces -------
            with tc.tile_pool(name="ppp", bufs=1, space="PSUM") as ppp:
                ps_lt = ppp.tile([R, 1], f32, tag="lt")
                for u in range(NU):
                    nc.tensor.matmul(out=ps_lt[:], lhsT=lat_sb[:, u, :],
                                     rhs=memb_sb[:, u:u + 1],
                                     start=(u == 0), stop=(u == NU - 1))
                lt2_sb = const.tile([R, 1], bf16)
                nc.vector.tensor_scalar_mul(lt2_sb[:], ps_lt[:], 2.0)  # alpha/r
                ps_pj = ppp.tile([1, FC], f32, tag="pj")
                for u in range(NU):
                    nc.tensor.matmul(
                        out=ps_pj[:], lhsT=memb_sb[:, u:u + 1],
                        rhs=proj_sb[:, u, :], start=(u == 0), stop=False)
                nc.tensor.matmul(out=ps_pj[:], lhsT=lt2_sb[:], rhs=lbt_sb[:],
                                 start=False, stop=True)
                off_sb = const.tile([1, FC], f32)
                nc.vector.tensor_add(off_sb[:], ps_pj[:], projb_sb[:])
            ftout_sb = const.tile([4, FC], f32)
            with tc.tile_pool(name="ppf", bufs=1, space="PSUM") as ppf:
                ps_ft = ppf.tile([4, FC], f32, tag="ft")
                nc.tensor.matmul(out=ps_ft[:], lhsT=ones1_sb[0:1, 0:4],
                                 rhs=off_sb[:], start=True, stop=True)
                nc.vector.tensor_tensor(out=ftout_sb[:], in0=ftemb_sb[:],
                                        in1=ps_ft[:], op=AT.add)
            nc.sync.dma_start(ft_out[:], ftout_sb[:])

    nc.compile()
    return nc


def _prep_inputs(inputs):
    import ml_dtypes
    bf16 = ml_dtypes.bfloat16
    fp8 = ml_dtypes.float8_e4m3

    nf = np.asarray(inputs["node_features"], dtype=np.float32)
    ids = np.asarray(inputs["input_ids"], dtype=np.int32)
    ids_flat = ids.reshape(-1)
    gw = np.asarray(inputs["gat_w"], dtype=np.float32)
    att_src = np.asarray(inputs["att_src"], dtype=np.float32)
    att_dst = np.asarray(inputs["att_dst"], dtype=np.float32)
    gbias = np.asarray(inputs["gat_bias"], dtype=np.float32)
    gamma = np.asarray(inputs["ln_gamma"], dtype=np.float32)
    beta = np.asarray(inputs["ln_beta"], dtype=np.float32)
    pw = np.asarray(inputs["proj_w"], dtype=np.float32)
    pb = np.asarray(inputs["proj_b"], dtype=np.float32)
    la = np.asarray(inputs["lora_a"], dtype=np.float32)
    lb = np.asarray(inputs["lora_b"], dtype=np.float32)
    emb = np.ascontiguousarray(np.asarray(inputs["embed"], dtype=np.float32).astype(bf16))

    def chunked(vec, parts=2):  # [parts*128] -> [128, parts]
        return np.ascontiguousarray(vec.reshape(parts, 128).T)

    def pre3(m, inner):  # [NU*128, inner] -> [128, NU, inner]
        return np.ascontiguousarray(
            m.reshape(NU, 128, inner).transpose(1, 0, 2).astype(bf16))

    att_strips = np.zeros((NT, 128, 2 * H), dtype=np.float32)
    for t in range(NT):
        h, u = t // NU, t % NU
        att_strips[t, :, h] = att_src[h, u * 128:(u + 1) * 128]
        att_strips[t, :, H + h] = att_dst[h, u * 128:(u + 1) * 128]
    att_st = np.ascontiguousarray(
        att_strips.transpose(1, 0, 2).astype(bf16))  # [128, NT, 2H]
    lora_a_pre = pre3(la.T, R)
    gamma_r = chunked(gamma, NU)
    beta_r = chunked(beta, NU)

    in_maps = []
    for c in range(NCORES):
        ech = slice(c * EC, (c + 1) * EC)
        fch = slice(c * FC, (c + 1) * FC)
        w_sl = gw[:, ech]
        nf_sl = nf[:, ech]
        m = {
            "w_nat": np.ascontiguousarray(w_sl.reshape(NT, 128, EC).transpose(1, 0, 2).astype(bf16)),
            "w_tr": np.ascontiguousarray(w_sl.T.astype(bf16)),
            "nf_pre": pre3(nf_sl, EC),
            "nf_tr": np.ascontiguousarray(nf_sl.T.astype(bf16)),
            "att_st": att_st,
            "proj_pre": pre3(pw[fch, :].T, FC),
            "projb_r": np.ascontiguousarray(pb[fch].reshape(1, FC)),
            "lora_a_pre": lora_a_pre,
            "lora_bt": np.ascontiguousarray(lb[fch, :].T.astype(bf16)),
            "gbias_row": (gbias.reshape(1, D).astype(np.float32)
                          if c == 0 else np.zeros((1, D), dtype=np.float32)),
            "gamma_r": gamma_r,
            "beta_r": beta_r,
            "ids_r": np.ascontiguousarray(
                ids_flat[c * ROWS:(c + 1) * ROWS].reshape(NG, 128).T),
            "ids_ft": np.ascontiguousarray(
                (ids[:, 0].astype(np.int64) * NCORES + c).astype(np.int32)
                .reshape(4, 1)),
            "embed": emb,
        }
        in_maps.append(m)
    return in_maps


def kernel(**inputs):
    _install_ntff_shim()
    from concourse.bass_utils import run_bass_kernel_spmd

    if "nc" not in _CACHE:
        _CACHE["nc"] = _build()
    nc = _CACHE["nc"]

    in_maps = _prep_inputs(inputs)
    trace = bool(int(os.environ.get("KERNEL_TRACE", "0")))
    res = run_bass_kernel_spmd(nc, in_maps, core_ids=list(range(NCORES)),
                               trace=trace)
    if trace:
        _CACHE["last_result"] = res
        print(f"HW exec time: {res.exec_time_ns} ns", flush=True)

    out = np.concatenate([res.results[c]["out_sl"] for c in range(NCORES)],
                         axis=0).astype(np.float32)
    out = out.reshape(B, S, D)
    ft = np.concatenate([res.results[c]["ft_out"] for c in range(NCORES)], axis=1)
    out[:, 0, :] = ft
    return out
